# revision 21
# baseline (speedup 1.0000x reference)
"""Trainium2 Bass kernel for nn_DebedderNeuron (scatter_memory).

Strategy: data-parallel over batch (16 rows per core x 8 cores).
The scatter-add in the reference has closed-form structure:
  y[b] = concat(L0w, L0bias, 0.5*(L1w_own + L0ext^T), 0.5*L1bias,
                L2w_own + L1ext^T, L2bias)

v5 design:
- All matmuls fp16 (x and W converted host-side); PSUM accumulation f32.
- Own-slice GEMMs orientation A (out = [kernel-part, w-cols]) with
  9-aligned PSUM chunks; extension GEMMs orientation B
  (out = [next-kernel-part, (b, prev-kernel)]).
- Ext results land in tmp SBUF tiles stored FLAT j-minor
  ((b*K + kd)*9 + j) so the PE fold below streams them contiguously.
  Drains from ext PSUM add the partition-varying bias and do the
  j-interleave via strided writes, split across scalar ACT (h=0/even-j)
  and vector tensor_scalar (h=1/odd-j). The free-varying (own-layer)
  bias is one flat fp16 dual-pump vector add per phase.
- The own+ext combine is FOLDED INTO THE PE: an identity-stationary
  matmul accumulates the tmp tile into the own-GEMM PSUM (contiguous
  rhs stream), so no per-row vector combines exist at all. A contiguous
  scalar ACT copy drains PSUM->SBUF for the store DMA.
- Schedule is input-bandwidth aware: l01 (smallest inputs) runs first
  while the heavy w1x/w2/xt2 stream; 24 identity warmup matmuls flip
  HAM to K=8/8 during the load window. l01(1) is hoisted before the
  last own_l2 so the post-PE tail is one row's drain+store.
- Output stores are full-128-partition DMAs alternating sync/gpsimd.
"""
import sys

if '/opt/trn_rl_repo' not in sys.path:
    sys.path.insert(0, '/opt/trn_rl_repo')

import numpy as np

N_CORES = 8
B = 128
BL = B // N_CORES          # 16 batch rows per core
D = 512
KS = 9
I_DIM = 370816
# y layout offsets
OFF_L0W, OFF_L0B = 0, 1728
OFF_L1W, OFF_L1B = 1792, 75520
OFF_L2W, OFF_L2B = 75648, 370560

# 9-aligned own-GEMM chunking: (psum offset, w-col start, n cols, n kd)
L2_CHUNKS = ((0, 0, 504, 56), (512, 504, 504, 56), (1024, 1008, 144, 16))
L1_CHUNKS = ((0, 0, 504, 56), (512, 504, 72, 8))

_CACHE = {}
_BIAS_CONSTS = [0.0, 0.0]  # (b2[1152], 0.5*b1[576]) — set before _build()


def _build():
    import concourse.bacc as bacc
    import concourse.mybir as mybir
    import concourse.tile as tile

    F32 = mybir.dt.float32
    F16 = mybir.dt.float16
    IDENT = mybir.ActivationFunctionType.Identity

    nc = bacc.Bacc("TRN2", target_bir_lowering=False, debug=False)

    # x pre-transposed on host: [k(4), p(128), cols] fp16, cols b-major
    xt0_d = nc.dram_tensor("xt0", [4, 128, BL * 64], F16, kind="ExternalInput").ap()
    xt1_d = nc.dram_tensor("xt1", [4, 128, BL * 128], F16, kind="ExternalInput").ap()
    xt2_d = nc.dram_tensor("xt2", [4, 128, BL * 256], F16, kind="ExternalInput").ap()
    w0_d = nc.dram_tensor("w0", [D, 28], F16, kind="ExternalInput").ap()
    w0x_d = nc.dram_tensor("w0x", [D, KS * 128], F16, kind="ExternalInput").ap()
    w1o_d = nc.dram_tensor("w1o", [D, 578], F16, kind="ExternalInput").ap()
    w1x_d = nc.dram_tensor("w1x", [D, KS * 256], F16, kind="ExternalInput").ap()
    w2_d = nc.dram_tensor("w2", [D, 1154], F16, kind="ExternalInput").ap()
    ident_d = nc.dram_tensor("ident", [128, 128], F16, kind="ExternalInput").ap()
    b0s_d = nc.dram_tensor("b0s", [128, 28], F32, kind="ExternalInput").ap()
    # per-partition bias columns for the ext drains
    b1e_d = nc.dram_tensor("b1e", [128, 2, KS], F32, kind="ExternalInput").ap()
    b0e_d = nc.dram_tensor("b0e", [128, KS], F32, kind="ExternalInput").ap()
    # free-varying (own-layer) bias patterns, flat (kd*9+j), un-replicated
    b2n_d = nc.dram_tensor("b2n", [128, 1152], F16, kind="ExternalInput").ap()
    b1n_d = nc.dram_tensor("b1n", [128, 576], F16, kind="ExternalInput").ap()
    y_d = nc.dram_tensor("y", [BL, I_DIM], F32, kind="ExternalOutput").ap()

    with tile.TileContext(nc) as tc:
        with tc.tile_pool(name="const", bufs=1) as cp, \
             tc.tile_pool(name="xts", bufs=2) as xtp, \
             tc.tile_pool(name="xt1s", bufs=2) as xt1p, \
             tc.tile_pool(name="xt0p", bufs=1) as xt0p, \
             tc.tile_pool(name="tmp", bufs=1) as tmpp, \
             tc.tile_pool(name="y2s", bufs=4) as y2p, \
             tc.tile_pool(name="y1s", bufs=2) as y1p, \
             tc.tile_pool(name="l0s", bufs=2) as l0p, \
             tc.tile_pool(name="pso", bufs=2, space="PSUM") as psop, \
             tc.tile_pool(name="pse", bufs=2, space="PSUM") as psep:

            ident = cp.tile([128, 128], F16, tag="ident")
            w0 = cp.tile([128, 4, 28], F16, tag="w0")
            w0x = cp.tile([128, 4, KS * 128], F16, tag="w0x")
            w1o = cp.tile([128, 4, 578], F16, tag="w1o")
            w1x = cp.tile([128, 4, KS * 256], F16, tag="w1x")
            w2 = cp.tile([128, 4, 1154], F16, tag="w2")
            b0s = cp.tile([128, 28], F32, tag="b0s")
            b1e = cp.tile([128, 2, KS], F32, tag="b1e")
            b0e = cp.tile([128, KS], F32, tag="b0e")
            b2n = cp.tile([128, 1152], F16, tag="b2n")
            b1n = cp.tile([128, 576], F16, tag="b1n")
            # b-replicated flat bias tiles (built on-device, vector copies)
            b2r = cp.tile([128, 8, 1152], F16, tag="b2r")
            b1r = cp.tile([128, 8, 576], F16, tag="b1r")

            l0b_all = cp.tile([128, 8], F32, tag="l0b")      # [(q,kd), pair]
            l1b_all = cp.tile([128, 16], F32, tag="l1b")     # [kn1, b]
            l2b_all = cp.tile([128, 2, 16], F32, tag="l2b")  # [kn2, t, b]

            w1xr = w1x_d.rearrange("(a p) l -> p a l", p=128)
            w2r = w2_d.rearrange("(a p) l -> p a l", p=128)
            w0xr = w0x_d.rearrange("(a p) l -> p a l", p=128)

            engs3 = (nc.sync, nc.scalar, nc.gpsimd)
            xt0t = xt0p.tile([128, 4, 1024], F16, tag="xt0", name="xt_xt0")
            xt1t0 = xt1p.tile([128, 4, 1024], F16, tag="xt1", name="xt_xt1")

            # ident first (tiny), then PE warmup matmuls: dummy PE work
            # during the load window flips HAM to K=8/8 before real MMs
            nc.sync.dma_start(ident[:, :], ident_d)
            ps_w = psep.tile([128, 512], F32, tag="pse", name="ps_w")
            for _ in range(24):
                nc.tensor.matmul(ps_w[:, 0:128], ident, ident,
                                 start=True, stop=True)

            # --- loads ordered by phase order (l01(0) first: cheapest
            # inputs), round-robin across 3 issue queues ---
            i = 0
            # A: ext0-su0 minimal (w0x j-triple 0, xt0 su0 half)
            for k in range(4):
                engs3[i % 3].dma_start(w0x[:, k, 0:384],
                                       w0xr[:, k, 0:384]); i += 1
                engs3[i % 3].dma_start(xt0t[:, k, 0:512],
                                       xt0_d[k, :, 0:512]); i += 1
            nc.sync.dma_start(b0e[:, :], b0e_d)
            nc.scalar.dma_start(b1n[:, :], b1n_d)
            nc.gpsimd.dma_start(b0s[:, :], b0s_d)
            for h in range(8):
                nc.vector.tensor_copy(b1r[:, h, :], b1n[:, :])
            # A2: rest of w0x
            for k in range(4):
                engs3[i % 3].dma_start(w0x[:, k, 384:768],
                                       w0xr[:, k, 384:768]); i += 1
                engs3[i % 3].dma_start(w0x[:, k, 768:1152],
                                       w0xr[:, k, 768:1152]); i += 1
            # B: l0 + own-L1 su0 inputs
            for k in range(4):
                engs3[i % 3].dma_start(
                    w0[:, k, :],
                    w0_d.rearrange("(a p) l -> p a l", p=128)[:, k, :]); i += 1
                engs3[i % 3].dma_start(
                    w1o[:, k, :],
                    w1o_d.rearrange("(a p) l -> p a l", p=128)[:, k, :]); i += 1
                engs3[i % 3].dma_start(xt1t0[:, k, :],
                                       xt1_d[k, :, 0:1024]); i += 1
            # C: ext1 inputs (w1x) + its biases
            for k in range(4):
                for jc in range(3):
                    engs3[i % 3].dma_start(
                        w1x[:, k, jc * 768:(jc + 1) * 768],
                        w1xr[:, k, jc * 768:(jc + 1) * 768]); i += 1
            nc.sync.dma_start(b1e[:, :, :], b1e_d)
            nc.scalar.dma_start(b2n[:, :], b2n_d)
            for h in range(8):
                nc.vector.tensor_copy(b2r[:, h, :], b2n[:, :])

            def load_xt2(su, t):
                """[128, 4, 1024] = (8b x 128 kn2) for kn2-half t (strided)."""
                t_ = xtp.tile([128, 4, 1024], F16, tag="xt2", name="xt_xt2")
                for k in range(4):
                    eng = (nc.sync, nc.scalar)[k % 2]
                    eng.dma_start(
                        t_[:, k, :].rearrange("p (b c) -> p b c", c=128),
                        xt2_d[k, :, su * 2048:(su + 1) * 2048].rearrange(
                            "p (b c) -> p b c", c=256)[:, :, t * 128:(t + 1) * 128])
                return t_

            def load_xt1(su):
                t_ = xt1p.tile([128, 4, 1024], F16, tag="xt1", name="xt_xt1")
                for k in range(4):
                    eng = (nc.sync, nc.scalar)[k % 2]
                    eng.dma_start(t_[:, k, :], xt1_d[k, :, su * 1024:(su + 1) * 1024])
                return t_

            def ext1_phase(su, t, xt1t):
                """ext1 GEMMs for 8 rows, half t -> tmp2 flat (b,kd,j) fp16."""
                tmp2 = tmpp.tile([128, 8, 128, KS], F16, tag="tmp2",
                                 name="tmp2", bufs=2)
                for j in range(KS):
                    for h in range(2):
                        ps = psep.tile([128, 512], F32, tag="pse")
                        for k in range(4):
                            nc.tensor.matmul(
                                ps[:, :],
                                w1x[:, k, j * 256 + t * 128:j * 256 + t * 128 + 128],
                                xt1t[:, k, h * 512:(h + 1) * 512],
                                start=(k == 0), stop=(k == 3))
                        # drain + partition-bias, j-interleaved (strided)
                        # write; split scalar/vector by h
                        dst = tmp2[:, h * 4:(h + 1) * 4, :, j]
                        src = ps[:, :].rearrange("p (b kd) -> p b kd", kd=128)
                        if h == 0:
                            nc.scalar.activation(dst, src, IDENT,
                                                 bias=b1e[:, t, j:j + 1])
                        else:
                            nc.vector.tensor_scalar_add(dst, src,
                                                        b1e[:, t, j:j + 1])
                # free-varying bias: one flat fp16 dual-pump add
                t2f = tmp2.rearrange("p b kd j -> p (b kd j)")
                nc.vector.tensor_add(t2f, t2f,
                                     b2r.rearrange("p b c -> p (b c)"))
                return tmp2

            def own_l2(su, t, xt2t, tmp2, b2_last):
                """own L2 GEMMs + PE-folded ext add + y2 DMA, 8 rows, half t."""
                t2f = tmp2.rearrange("p b kd j -> p (b kd j)")
                for bq in range(8):
                    gb = su * 8 + bq
                    y2sb = y2p.tile([128, 1154], F32, tag="y2s", name="y2sb")
                    pso = psop.tile([128, 1536], F32, tag="pso")
                    for k in range(4):
                        st_ = xt2t[:, k, bq * 128:(bq + 1) * 128]
                        for (po, c0, ncc, nkd) in L2_CHUNKS:
                            n = ncc + (2 if c0 == 1008 else 0)  # chunk3 incl bias col
                            nc.tensor.matmul(pso[:, po:po + n], st_,
                                             w2[:, k, c0:c0 + n],
                                             start=(k == 0), stop=False)
                    # fold ext+biases into PSUM: identity-stationary matmul
                    # streams the flat tmp2 slice (contiguous) accumulating
                    # onto the own-GEMM result — no vector combine at all
                    for ci, (po, c0, ncc, nkd) in enumerate(L2_CHUNKS):
                        nc.tensor.matmul(
                            pso[:, po:po + ncc], ident,
                            t2f[:, bq * 1152 + c0:bq * 1152 + c0 + ncc],
                            start=False, stop=(ci == 2),
                            skip_group_check=True)
                    # contiguous drains PSUM -> SBUF for the store
                    nc.scalar.copy(
                        y2sb[:, 0:1008].rearrange("p (c e) -> p c e", e=504),
                        pso[:, 0:1024].rearrange(
                            "p (c e) -> p c e", e=512)[:, :, 0:504])
                    nc.scalar.copy(y2sb[:, 1008:1154], pso[:, 1024:1170])
                    nc.vector.tensor_scalar_add(
                        l2b_all[:, t, gb:gb + 1], pso[:, 1168:1169], b2_last)
                    # one full-128-partition DMA (contiguous 576KB HBM span);
                    # alternate issue queues to parallelize descriptor gen
                    eng = (nc.sync, nc.gpsimd)[bq % 2]
                    eng.dma_start(
                        y_d[gb, OFF_L2W + t * 147456:
                            OFF_L2W + (t + 1) * 147456]
                        .rearrange("(kn w) -> kn w", w=1152),
                        y2sb[:, 0:1152])
                return

            def l01_phase(su, xt1t, b1_last):
                """ext0 + own L1 + own L0 for 8 rows."""
                # ext0: j loop, out [kn1, (8b, 64kd0)] -> tmp1 flat (b,kd,j)
                tmp1 = tmpp.tile([128, 8, 64, KS], F16, tag="tmp1", name="tmp1")
                for j in range(KS):
                    ps = psep.tile([128, 512], F32, tag="pse")
                    for k in range(4):
                        nc.tensor.matmul(ps[:, :],
                                         w0x[:, k, j * 128:(j + 1) * 128],
                                         xt0t[:, k, su * 512:(su + 1) * 512],
                                         start=(k == 0), stop=(k == 3))
                    dst = tmp1[:, :, :, j]
                    src = ps[:, :].rearrange("p (b kd) -> p b kd", kd=64)
                    if j % 2 == 0:
                        nc.scalar.activation(dst, src, IDENT,
                                             bias=b0e[:, j:j + 1])
                    else:
                        nc.vector.tensor_scalar_add(dst, src, b0e[:, j:j + 1])
                t1f = tmp1.rearrange("p b kd j -> p (b kd j)")
                nc.vector.tensor_add(t1f, t1f,
                                     b1r.rearrange("p b c -> p (b c)"))
                # own L1 per row + PE-folded ext0 add
                for bq in range(8):
                    gb = su * 8 + bq
                    if bq % 4 == 0:
                        y1sb = y1p.tile([128, 4, 578], F32, tag="y1s", name="y1sb")
                    pso = psop.tile([128, 1536], F32, tag="pso")
                    for k in range(4):
                        st_ = xt1t[:, k, bq * 128:(bq + 1) * 128]
                        for (po, c0, ncc, nkd) in L1_CHUNKS:
                            n = ncc + (2 if c0 == 504 else 0)  # incl bias col + pad
                            nc.tensor.matmul(pso[:, po:po + n], st_,
                                             w1o[:, k, c0:c0 + n],
                                             start=(k == 0), stop=False)
                    for ci, (po, c0, ncc, nkd) in enumerate(L1_CHUNKS):
                        nc.tensor.matmul(
                            pso[:, po:po + ncc], ident,
                            t1f[:, bq * 576 + c0:bq * 576 + c0 + ncc],
                            start=False, stop=(ci == 1),
                            skip_group_check=True)
                    nc.scalar.copy(y1sb[:, bq % 4, 0:504], pso[:, 0:504])
                    nc.scalar.copy(y1sb[:, bq % 4, 504:576], pso[:, 512:584])
                    nc.vector.tensor_scalar_add(
                        l1b_all[:, gb:gb + 1], pso[:, 584:585], b1_last)
                    if bq % 4 == 3:
                        b0r = su * 8 + (bq - 3)
                        nc.gpsimd.dma_start(
                            y_d[b0r:b0r + 4, OFF_L1W:OFF_L1B]
                            .rearrange("b (kn w) -> kn b w", w=576),
                            y1sb[:, :, 0:576])
                # own L0 per pair of rows
                l0sb = l0p.tile([128, 4, 27], F32, tag="l0s", name="l0sb")
                for pair in range(4):
                    ps = psep.tile([128, 512], F32, tag="pse")
                    for k in range(4):
                        nc.tensor.matmul(
                            ps[:, 0:28],
                            xt0t[:, k, su * 512 + pair * 128:su * 512 + (pair + 1) * 128],
                            w0[:, k, 0:28],
                            start=(k == 0), stop=(k == 3))
                    nc.vector.tensor_add(l0sb[:, pair, :], ps[:, 0:27], b0s[:, 0:27])
                    gp = 4 * su + pair
                    nc.vector.tensor_add(l0b_all[:, gp:gp + 1], ps[:, 27:28],
                                         b0s[:, 27:28])
                for q in range(2):
                    nc.sync.dma_start(
                        y_d[su * 8:su * 8 + 8, OFF_L0W:OFF_L0B]
                        .rearrange("(p q) (kd w) -> q kd p w", q=2, w=27)[q],
                        l0sb[q * 64:(q + 1) * 64, :, :])

            # ---- per-su bias-region stores: transposing DMAs (partition
            # dim lands contiguous in HBM), no PE/vector involvement ----
            def finals(su):
                for q in range(2):
                    nc.sync.dma_start(
                        y_d[su * 8:su * 8 + 8, OFF_L0B:OFF_L1W]
                        .rearrange("(p q) c -> q c p", q=2)[q],
                        l0b_all[q * 64:(q + 1) * 64, su * 4:su * 4 + 4])
                nc.sync.dma_start(
                    y_d[su * 8:su * 8 + 8, OFF_L1B:OFF_L2W]
                    .rearrange("b c -> c b"),
                    l1b_all[:, su * 8:su * 8 + 8])
                for t in range(2):
                    nc.sync.dma_start(
                        y_d[su * 8:su * 8 + 8,
                            OFF_L2B + t * 128:OFF_L2B + (t + 1) * 128]
                        .rearrange("b c -> c b"),
                        l2b_all[:, t, su * 8:su * 8 + 8])

            # ---------------- main schedule ----------------
            b2_last, b1_last = _BIAS_CONSTS

            # su0's heavy inputs: issue before l01 so rings stream during it
            xt2t00 = load_xt2(0, 0)
            for k in range(4):
                eng = (nc.sync, nc.scalar)[k % 2]
                eng.dma_start(w2[:, k, :], w2r[:, k, :])
            xt2t01 = load_xt2(0, 1)
            # xt0 su1-half (needed only at l01(1))
            for k in range(4):
                engs3[k % 3].dma_start(xt0t[:, k, 512:1024],
                                       xt0_d[k, :, 512:1024])

            l01_phase(0, xt1t0, b1_last)
            tmp2 = ext1_phase(0, 0, xt1t0)
            xt1t1 = load_xt1(1)
            own_l2(0, 0, xt2t00, tmp2, b2_last)
            tmp2 = ext1_phase(0, 1, xt1t0)
            xt2t10 = load_xt2(1, 0)
            own_l2(0, 1, xt2t01, tmp2, b2_last)
            finals(0)
            tmp2 = ext1_phase(1, 0, xt1t1)
            xt2t11 = load_xt2(1, 1)
            own_l2(1, 0, xt2t10, tmp2, b2_last)
            tmp2 = ext1_phase(1, 1, xt1t1)
            l01_phase(1, xt1t1, b1_last)
            own_l2(1, 1, xt2t11, tmp2, b2_last)
            finals(1)

    nc.compile()
    return nc


def _prep_shared(W0, b0, W1, b1, W2, b2):
    """Host-side prescale + bias tile construction (numpy, core-independent)."""
    f16 = np.float16
    W0own = W0[:, :28].astype(f16)
    # ext0 cols packed dense, j-major: w0x[:, j*128 + kn] = 0.5*W0[:, 28+kn*9+j]
    W0x = (0.5 * W0[:, 28:]).reshape(D, 128, KS).transpose(0, 2, 1).reshape(
        D, KS * 128).astype(f16)
    W1o = np.zeros((D, 578), f16)
    W1o[:, :577] = (0.5 * W1[:, :577]).astype(f16)
    # ext1 cols packed dense, j-major: w1x[:, j*256 + kn] = W1[:, 577+kn*9+j]
    W1x = W1[:, 577:].reshape(D, 256, KS).transpose(0, 2, 1).reshape(
        D, KS * 256).astype(f16)
    W2p = np.zeros((D, 1154), f16)
    W2p[:, :1153] = W2.astype(f16)

    b0s = np.tile(b0[None, :28], (128, 1)).astype(np.float32)
    # per-partition (drain) bias columns
    b1e = np.zeros((128, 2, KS), np.float32)          # b1[577 + kn2*9 + j]
    for t in range(2):
        b1e[:, t, :] = b1[577:].reshape(256, KS)[t * 128:(t + 1) * 128]
    b0e = (0.5 * b0[28:1180].reshape(128, KS)).astype(np.float32)  # [kn1, j]
    # free-varying (own-layer) bias patterns, flat (kd*9+j), part-replicated
    b2n = np.tile(b2[:1152].astype(f16)[None, :], (128, 1))
    b1n = np.tile((0.5 * b1[:576]).astype(f16)[None, :], (128, 1))

    return (W0own, W0x, W1o, W1x, W2p, b0s, b1e, b0e, b2n, b1n,
            float(b2[1152]), float(0.5 * b1[576]))


def kernel(x, W0, b0, W1, b1, W2, b2, _trace=False):
    from concourse import bass_utils

    x = np.asarray(x, np.float32)
    (W0own, W0x, W1o, W1x, W2p, b0s, b1e, b0e, b2n, b1n,
     b2_last, b1_last) = _prep_shared(
        np.asarray(W0, np.float32), np.asarray(b0, np.float32),
        np.asarray(W1, np.float32), np.asarray(b1, np.float32),
        np.asarray(W2, np.float32), np.asarray(b2, np.float32))

    if "nc" not in _CACHE:
        # bias-column constants are baked into the program as immediates
        _BIAS_CONSTS[0] = b2_last
        _BIAS_CONSTS[1] = b1_last
        _CACHE["nc"] = _build()
    nc = _CACHE["nc"]

    ident = np.eye(128, dtype=np.float16)

    # shard + transpose x on host: [B,448,512] -> per-core d-major fp16 layouts
    xs = x.reshape(N_CORES, BL, 448, D)
    in_maps = []
    for c in range(N_CORES):
        xc = xs[c]  # [BL, 448, 512]
        xt0 = np.ascontiguousarray(
            xc[:, 0:64, :].transpose(2, 0, 1)).reshape(4, 128, BL * 64).astype(
            np.float16)
        xt1 = np.ascontiguousarray(
            xc[:, 64:192, :].transpose(2, 0, 1)).reshape(4, 128, BL * 128).astype(
            np.float16)
        xt2 = np.ascontiguousarray(
            xc[:, 192:448, :].transpose(2, 0, 1)).reshape(4, 128, BL * 256).astype(
            np.float16)
        in_maps.append({
            "xt0": xt0, "xt1": xt1, "xt2": xt2,
            "w0": W0own, "w0x": W0x, "w1o": W1o, "w1x": W1x, "w2": W2p,
            "b0s": b0s, "b1e": b1e, "b0e": b0e, "b2n": b2n, "b1n": b1n,
            "ident": ident,
        })

    res = bass_utils.run_bass_kernel_spmd(
        nc, in_maps, core_ids=list(range(N_CORES)), trace=_trace)
    _CACHE["last_res"] = res
    y = np.concatenate([res.results[c]["y"] for c in range(N_CORES)], axis=0)
    return y


# revision 25
# speedup vs baseline: 1.1936x; 1.1936x over previous
"""Trainium2 Bass kernel for nn_DebedderNeuron (scatter_memory).

Strategy: data-parallel over batch (16 rows per core x 8 cores).
The scatter-add in the reference has closed-form structure:
  y[b] = concat(L0w, L0bias, 0.5*(L1w_own + L0ext^T), 0.5*L1bias,
                L2w_own + L1ext^T, L2bias)

v5 design:
- All matmuls fp16 (x and W converted host-side); PSUM accumulation f32.
- Own-slice GEMMs orientation A (out = [kernel-part, w-cols]) with
  9-aligned PSUM chunks; extension GEMMs orientation B
  (out = [next-kernel-part, (b, prev-kernel)]).
- Ext results land in tmp SBUF tiles stored FLAT j-minor
  ((b*K + kd)*9 + j) so the PE fold below streams them contiguously.
  Drains from ext PSUM add the partition-varying bias and do the
  j-interleave via strided writes, split across scalar ACT (h=0/even-j)
  and vector tensor_scalar (h=1/odd-j). The free-varying (own-layer)
  bias is one flat fp16 dual-pump vector add per phase.
- The own+ext combine is FOLDED INTO THE PE: an identity-stationary
  matmul accumulates the tmp tile into the own-GEMM PSUM (contiguous
  rhs stream), so no per-row vector combines exist at all. A contiguous
  scalar ACT copy drains PSUM->SBUF for the store DMA.
- Schedule is input-bandwidth aware: l01 (smallest inputs) runs first
  while the heavy w1x/w2/xt2 stream; 24 identity warmup matmuls flip
  HAM to K=8/8 during the load window. l01(1) is hoisted before the
  last own_l2 so the post-PE tail is one row's drain+store.
- Output stores are full-128-partition DMAs alternating sync/gpsimd.
"""
import sys

if '/opt/trn_rl_repo' not in sys.path:
    sys.path.insert(0, '/opt/trn_rl_repo')

import numpy as np

N_CORES = 8
B = 128
BL = B // N_CORES          # 16 batch rows per core
D = 512
KS = 9
I_DIM = 370816
# y layout offsets
OFF_L0W, OFF_L0B = 0, 1728
OFF_L1W, OFF_L1B = 1792, 75520
OFF_L2W, OFF_L2B = 75648, 370560

# 9-aligned own-GEMM chunking: (psum offset, w-col start, n cols, n kd)
L2_CHUNKS = ((0, 0, 504, 56), (512, 504, 504, 56), (1024, 1008, 144, 16))
L1_CHUNKS = ((0, 0, 504, 56), (512, 504, 72, 8))

_CACHE = {}
_BIAS_CONSTS = [0.0, 0.0]  # (b2[1152], 0.5*b1[576]) — set before _build()


def _build():
    import concourse.bacc as bacc
    import concourse.mybir as mybir
    import concourse.tile as tile

    F32 = mybir.dt.float32
    F16 = mybir.dt.float16
    IDENT = mybir.ActivationFunctionType.Identity

    nc = bacc.Bacc("TRN2", target_bir_lowering=False, debug=False)

    # x pre-transposed on host: [k(4), p(128), cols] fp16, cols b-major
    xt0_d = nc.dram_tensor("xt0", [4, 128, BL * 64], F16, kind="ExternalInput").ap()
    xt1_d = nc.dram_tensor("xt1", [4, 128, BL * 128], F16, kind="ExternalInput").ap()
    xt2_d = nc.dram_tensor("xt2", [4, 128, BL * 256], F16, kind="ExternalInput").ap()
    w0_d = nc.dram_tensor("w0", [D, 28], F16, kind="ExternalInput").ap()
    w0x_d = nc.dram_tensor("w0x", [D, KS * 128], F16, kind="ExternalInput").ap()
    w1o_d = nc.dram_tensor("w1o", [D, 578], F16, kind="ExternalInput").ap()
    w1x_d = nc.dram_tensor("w1x", [D, KS * 256], F16, kind="ExternalInput").ap()
    w2_d = nc.dram_tensor("w2", [D, 1154], F16, kind="ExternalInput").ap()
    ident_d = nc.dram_tensor("ident", [128, 128], F16, kind="ExternalInput").ap()
    b0s_d = nc.dram_tensor("b0s", [128, 28], F32, kind="ExternalInput").ap()
    # per-partition bias columns for the ext drains
    b1e_d = nc.dram_tensor("b1e", [128, 2, KS], F32, kind="ExternalInput").ap()
    b0e_d = nc.dram_tensor("b0e", [128, KS], F32, kind="ExternalInput").ap()
    # free-varying (own-layer) bias patterns, flat (kd*9+j), un-replicated
    b2n_d = nc.dram_tensor("b2n", [128, 1152], F16, kind="ExternalInput").ap()
    b1n_d = nc.dram_tensor("b1n", [128, 576], F16, kind="ExternalInput").ap()
    y_d = nc.dram_tensor("y", [BL, I_DIM], F32, kind="ExternalOutput").ap()

    with tile.TileContext(nc) as tc:
        with tc.tile_pool(name="const", bufs=1) as cp, \
             tc.tile_pool(name="xts", bufs=2) as xtp, \
             tc.tile_pool(name="xt1s", bufs=2) as xt1p, \
             tc.tile_pool(name="xt0p", bufs=1) as xt0p, \
             tc.tile_pool(name="tmp", bufs=1) as tmpp, \
             tc.tile_pool(name="y2s", bufs=3) as y2p, \
             tc.tile_pool(name="y1s", bufs=2) as y1p, \
             tc.tile_pool(name="l0s", bufs=2) as l0p, \
             tc.tile_pool(name="pso", bufs=2, space="PSUM") as psop, \
             tc.tile_pool(name="pse", bufs=2, space="PSUM") as psep:

            ident = cp.tile([128, 128], F16, tag="ident")
            w0 = cp.tile([128, 4, 28], F16, tag="w0")
            w0x = cp.tile([128, 4, KS * 128], F16, tag="w0x")
            w1o = cp.tile([128, 4, 578], F16, tag="w1o")
            w1x = cp.tile([128, 4, KS * 256], F16, tag="w1x")
            w2 = cp.tile([128, 4, 1154], F16, tag="w2")
            b0s = cp.tile([128, 28], F32, tag="b0s")
            b1e = cp.tile([128, 2, KS], F32, tag="b1e")
            b0e = cp.tile([128, KS], F32, tag="b0e")
            b2n = cp.tile([128, 1152], F16, tag="b2n")
            b1n = cp.tile([128, 576], F16, tag="b1n")
            # b-replicated flat bias tiles (built on-device, vector copies)
            b2r = cp.tile([128, 8, 1152], F16, tag="b2r")
            b1r = cp.tile([128, 8, 576], F16, tag="b1r")

            l0b_all = cp.tile([128, 8], F32, tag="l0b")      # [(q,kd), pair]
            l1b_all = cp.tile([128, 16], F32, tag="l1b")     # [kn1, b]
            l2b_all = cp.tile([128, 2, 16], F32, tag="l2b")  # [kn2, t, b]

            w1xr = w1x_d.rearrange("(a p) l -> p a l", p=128)
            w2r = w2_d.rearrange("(a p) l -> p a l", p=128)
            w0xr = w0x_d.rearrange("(a p) l -> p a l", p=128)

            engs3 = (nc.sync, nc.scalar, nc.gpsimd)
            xt0t = xt0p.tile([128, 4, 1024], F16, tag="xt0", name="xt_xt0")
            xt1t0 = xt1p.tile([128, 4, 1024], F16, tag="xt1", name="xt_xt1")

            # ident first (tiny), then PE warmup matmuls: dummy PE work
            # during the load window flips HAM to K=8/8 before real MMs
            nc.sync.dma_start(ident[:, :], ident_d)
            ps_w = psep.tile([128, 512], F32, tag="pse", name="ps_w")
            for _ in range(24):
                nc.tensor.matmul(ps_w[:, 0:128], ident, ident,
                                 start=True, stop=True)

            # --- loads ordered by phase order (l01(0) first: cheapest
            # inputs), round-robin across 3 issue queues ---
            i = 0
            # A: ext0-su0 minimal (w0x j-triple 0, xt0 su0 half)
            for k in range(4):
                engs3[i % 3].dma_start(w0x[:, k, 0:384],
                                       w0xr[:, k, 0:384]); i += 1
                engs3[i % 3].dma_start(xt0t[:, k, 0:512],
                                       xt0_d[k, :, 0:512]); i += 1
            nc.sync.dma_start(b0e[:, :], b0e_d)
            nc.scalar.dma_start(b1n[:, :], b1n_d)
            nc.gpsimd.dma_start(b0s[:, :], b0s_d)
            for h in range(8):
                nc.vector.tensor_copy(b1r[:, h, :], b1n[:, :])
            # A2: rest of w0x
            for k in range(4):
                engs3[i % 3].dma_start(w0x[:, k, 384:768],
                                       w0xr[:, k, 384:768]); i += 1
                engs3[i % 3].dma_start(w0x[:, k, 768:1152],
                                       w0xr[:, k, 768:1152]); i += 1
            # B: l0 + own-L1 su0 inputs
            for k in range(4):
                engs3[i % 3].dma_start(
                    w0[:, k, :],
                    w0_d.rearrange("(a p) l -> p a l", p=128)[:, k, :]); i += 1
                engs3[i % 3].dma_start(
                    w1o[:, k, :],
                    w1o_d.rearrange("(a p) l -> p a l", p=128)[:, k, :]); i += 1
                engs3[i % 3].dma_start(xt1t0[:, k, :],
                                       xt1_d[k, :, 0:1024]); i += 1
            # C: ext1 inputs (w1x) + its biases
            for k in range(4):
                for jc in range(3):
                    engs3[i % 3].dma_start(
                        w1x[:, k, jc * 768:(jc + 1) * 768],
                        w1xr[:, k, jc * 768:(jc + 1) * 768]); i += 1
            nc.sync.dma_start(b1e[:, :, :], b1e_d)
            nc.scalar.dma_start(b2n[:, :], b2n_d)
            for h in range(8):
                nc.vector.tensor_copy(b2r[:, h, :], b2n[:, :])

            def load_xt2(su, t):
                """[128, 4, 1024] = (8b x 128 kn2) for kn2-half t (strided)."""
                t_ = xtp.tile([128, 4, 1024], F16, tag="xt2", name="xt_xt2")
                for k in range(4):
                    eng = (nc.sync, nc.scalar)[k % 2]
                    eng.dma_start(
                        t_[:, k, :].rearrange("p (b c) -> p b c", c=128),
                        xt2_d[k, :, su * 2048:(su + 1) * 2048].rearrange(
                            "p (b c) -> p b c", c=256)[:, :, t * 128:(t + 1) * 128])
                return t_

            def load_xt1(su):
                t_ = xt1p.tile([128, 4, 1024], F16, tag="xt1", name="xt_xt1")
                for k in range(4):
                    eng = (nc.sync, nc.scalar)[k % 2]
                    eng.dma_start(t_[:, k, :], xt1_d[k, :, su * 1024:(su + 1) * 1024])
                return t_

            def ext1_phase(su, t, xt1t):
                """ext1 GEMMs for 8 rows, half t -> tmp2 flat (b,kd,j) fp16.

                h-outer loop: after each h-half's 9 contiguous ACT drains
                land in tmp2j, ONE 3D-AP op (strided read, sequential
                write) interleaves j and adds the free-varying bias,
                alternating vector/gpsimd so the two halves overlap.
                """
                tmp2 = tmpp.tile([128, 8, 128, KS], F16, tag="tmp2",
                                 name="tmp2", bufs=1)
                for h in range(2):
                    t2j = tmpp.tile([128, KS, 512], F16, tag="tmp2j",
                                    name="t2j", bufs=2)
                    for j in range(KS):
                        ps = psep.tile([128, 512], F32, tag="pse")
                        for k in range(4):
                            nc.tensor.matmul(
                                ps[:, :],
                                w1x[:, k, j * 256 + t * 128:j * 256 + t * 128 + 128],
                                xt1t[:, k, h * 512:(h + 1) * 512],
                                start=(k == 0), stop=(k == 3))
                        nc.scalar.activation(t2j[:, j, :], ps[:, :],
                                             IDENT, bias=b1e[:, t, j:j + 1])
                    eng = (nc.vector, nc.gpsimd)[h]
                    eng.tensor_add(
                        tmp2[:, h * 4:(h + 1) * 4, :, :].rearrange(
                            "p b kd j -> p (b kd) j"),
                        t2j.rearrange("p j bk -> p bk j"),
                        b2r[:, h * 4:(h + 1) * 4, :].rearrange(
                            "p b (kd j) -> p (b kd) j", j=KS))
                return tmp2

            def own_l2(su, t, xt2t, tmp2, b2_last):
                """own L2 GEMMs + PE-folded ext add + y2 DMA, 8 rows, half t."""
                t2f = tmp2.rearrange("p b kd j -> p (b kd j)")
                for bq in range(8):
                    gb = su * 8 + bq
                    y2sb = y2p.tile([128, 1154], F32, tag="y2s", name="y2sb")
                    pso = psop.tile([128, 1536], F32, tag="pso")
                    for k in range(4):
                        st_ = xt2t[:, k, bq * 128:(bq + 1) * 128]
                        for (po, c0, ncc, nkd) in L2_CHUNKS:
                            n = ncc + (2 if c0 == 1008 else 0)  # chunk3 incl bias col
                            nc.tensor.matmul(pso[:, po:po + n], st_,
                                             w2[:, k, c0:c0 + n],
                                             start=(k == 0), stop=False)
                    # fold ext+biases into PSUM: identity-stationary matmul
                    # streams the flat tmp2 slice (contiguous) accumulating
                    # onto the own-GEMM result — no vector combine at all
                    for ci, (po, c0, ncc, nkd) in enumerate(L2_CHUNKS):
                        nc.tensor.matmul(
                            pso[:, po:po + ncc], ident,
                            t2f[:, bq * 1152 + c0:bq * 1152 + c0 + ncc],
                            start=False, stop=(ci == 2),
                            skip_group_check=True)
                    # contiguous drains PSUM -> SBUF for the store
                    nc.scalar.copy(
                        y2sb[:, 0:1008].rearrange("p (c e) -> p c e", e=504),
                        pso[:, 0:1024].rearrange(
                            "p (c e) -> p c e", e=512)[:, :, 0:504])
                    nc.scalar.copy(y2sb[:, 1008:1154], pso[:, 1024:1170])
                    nc.vector.tensor_scalar_add(
                        l2b_all[:, t, gb:gb + 1], pso[:, 1168:1169], b2_last)
                    # one full-128-partition DMA (contiguous 576KB HBM span);
                    # alternate issue queues to parallelize descriptor gen
                    eng = (nc.sync, nc.gpsimd)[bq % 2]
                    eng.dma_start(
                        y_d[gb, OFF_L2W + t * 147456:
                            OFF_L2W + (t + 1) * 147456]
                        .rearrange("(kn w) -> kn w", w=1152),
                        y2sb[:, 0:1152])
                return

            def l01_phase(su, xt1t, b1_last):
                """ext0 + own L1 + own L0 for 8 rows."""
                # ext0: j loop, out [kn1, (8b, 64kd0)] -> tmp1 flat (b,kd,j)
                tmp1 = tmpp.tile([128, 8, 64, KS], F16, tag="tmp1", name="tmp1")
                t1j = tmpp.tile([128, KS, 512], F16, tag="tmp1j", name="t1j")
                for j in range(KS):
                    ps = psep.tile([128, 512], F32, tag="pse")
                    for k in range(4):
                        nc.tensor.matmul(ps[:, :],
                                         w0x[:, k, j * 128:(j + 1) * 128],
                                         xt0t[:, k, su * 512:(su + 1) * 512],
                                         start=(k == 0), stop=(k == 3))
                    nc.scalar.activation(t1j[:, j, :], ps[:, :],
                                         IDENT, bias=b0e[:, j:j + 1])
                eng = (nc.vector, nc.gpsimd)[su]
                eng.tensor_add(
                    tmp1.rearrange("p b kd j -> p (b kd) j"),
                    t1j.rearrange("p j bk -> p bk j"),
                    b1r.rearrange("p b (kd j) -> p (b kd) j", j=KS))
                t1f = tmp1.rearrange("p b kd j -> p (b kd j)")
                # own L1 per row + PE-folded ext0 add
                for bq in range(8):
                    gb = su * 8 + bq
                    if bq % 4 == 0:
                        y1sb = y1p.tile([128, 4, 578], F32, tag="y1s", name="y1sb")
                    pso = psop.tile([128, 1536], F32, tag="pso")
                    for k in range(4):
                        st_ = xt1t[:, k, bq * 128:(bq + 1) * 128]
                        for (po, c0, ncc, nkd) in L1_CHUNKS:
                            n = ncc + (2 if c0 == 504 else 0)  # incl bias col + pad
                            nc.tensor.matmul(pso[:, po:po + n], st_,
                                             w1o[:, k, c0:c0 + n],
                                             start=(k == 0), stop=False)
                    for ci, (po, c0, ncc, nkd) in enumerate(L1_CHUNKS):
                        nc.tensor.matmul(
                            pso[:, po:po + ncc], ident,
                            t1f[:, bq * 576 + c0:bq * 576 + c0 + ncc],
                            start=False, stop=(ci == 1),
                            skip_group_check=True)
                    nc.scalar.copy(y1sb[:, bq % 4, 0:504], pso[:, 0:504])
                    nc.scalar.copy(y1sb[:, bq % 4, 504:576], pso[:, 512:584])
                    nc.vector.tensor_scalar_add(
                        l1b_all[:, gb:gb + 1], pso[:, 584:585], b1_last)
                    if bq % 4 == 3:
                        b0r = su * 8 + (bq - 3)
                        nc.gpsimd.dma_start(
                            y_d[b0r:b0r + 4, OFF_L1W:OFF_L1B]
                            .rearrange("b (kn w) -> kn b w", w=576),
                            y1sb[:, :, 0:576])
                # own L0 per pair of rows
                l0sb = l0p.tile([128, 4, 27], F32, tag="l0s", name="l0sb")
                for pair in range(4):
                    ps = psep.tile([128, 512], F32, tag="pse")
                    for k in range(4):
                        nc.tensor.matmul(
                            ps[:, 0:28],
                            xt0t[:, k, su * 512 + pair * 128:su * 512 + (pair + 1) * 128],
                            w0[:, k, 0:28],
                            start=(k == 0), stop=(k == 3))
                    nc.vector.tensor_add(l0sb[:, pair, :], ps[:, 0:27], b0s[:, 0:27])
                    gp = 4 * su + pair
                    nc.vector.tensor_add(l0b_all[:, gp:gp + 1], ps[:, 27:28],
                                         b0s[:, 27:28])
                for q in range(2):
                    nc.sync.dma_start(
                        y_d[su * 8:su * 8 + 8, OFF_L0W:OFF_L0B]
                        .rearrange("(p q) (kd w) -> q kd p w", q=2, w=27)[q],
                        l0sb[q * 64:(q + 1) * 64, :, :])

            # ---- per-su bias-region stores: transposing DMAs (partition
            # dim lands contiguous in HBM), no PE/vector involvement ----
            def finals(su):
                for q in range(2):
                    nc.sync.dma_start(
                        y_d[su * 8:su * 8 + 8, OFF_L0B:OFF_L1W]
                        .rearrange("(p q) c -> q c p", q=2)[q],
                        l0b_all[q * 64:(q + 1) * 64, su * 4:su * 4 + 4])
                nc.sync.dma_start(
                    y_d[su * 8:su * 8 + 8, OFF_L1B:OFF_L2W]
                    .rearrange("b c -> c b"),
                    l1b_all[:, su * 8:su * 8 + 8])
                for t in range(2):
                    nc.sync.dma_start(
                        y_d[su * 8:su * 8 + 8,
                            OFF_L2B + t * 128:OFF_L2B + (t + 1) * 128]
                        .rearrange("b c -> c b"),
                        l2b_all[:, t, su * 8:su * 8 + 8])

            # ---------------- main schedule ----------------
            b2_last, b1_last = _BIAS_CONSTS

            # su0's heavy inputs: issue before l01 so rings stream during it
            xt2t00 = load_xt2(0, 0)
            for k in range(4):
                eng = (nc.sync, nc.scalar)[k % 2]
                eng.dma_start(w2[:, k, :], w2r[:, k, :])
            xt2t01 = load_xt2(0, 1)
            # xt0 su1-half (needed only at l01(1))
            for k in range(4):
                engs3[k % 3].dma_start(xt0t[:, k, 512:1024],
                                       xt0_d[k, :, 512:1024])

            l01_phase(0, xt1t0, b1_last)
            tmp2 = ext1_phase(0, 0, xt1t0)
            xt1t1 = load_xt1(1)
            own_l2(0, 0, xt2t00, tmp2, b2_last)
            tmp2 = ext1_phase(0, 1, xt1t0)
            xt2t10 = load_xt2(1, 0)
            own_l2(0, 1, xt2t01, tmp2, b2_last)
            finals(0)
            tmp2 = ext1_phase(1, 0, xt1t1)
            xt2t11 = load_xt2(1, 1)
            own_l2(1, 0, xt2t10, tmp2, b2_last)
            tmp2 = ext1_phase(1, 1, xt1t1)
            l01_phase(1, xt1t1, b1_last)
            own_l2(1, 1, xt2t11, tmp2, b2_last)
            finals(1)

    nc.compile()
    return nc


def _prep_shared(W0, b0, W1, b1, W2, b2):
    """Host-side prescale + bias tile construction (numpy, core-independent)."""
    f16 = np.float16
    W0own = W0[:, :28].astype(f16)
    # ext0 cols packed dense, j-major: w0x[:, j*128 + kn] = 0.5*W0[:, 28+kn*9+j]
    W0x = (0.5 * W0[:, 28:]).reshape(D, 128, KS).transpose(0, 2, 1).reshape(
        D, KS * 128).astype(f16)
    W1o = np.zeros((D, 578), f16)
    W1o[:, :577] = (0.5 * W1[:, :577]).astype(f16)
    # ext1 cols packed dense, j-major: w1x[:, j*256 + kn] = W1[:, 577+kn*9+j]
    W1x = W1[:, 577:].reshape(D, 256, KS).transpose(0, 2, 1).reshape(
        D, KS * 256).astype(f16)
    W2p = np.zeros((D, 1154), f16)
    W2p[:, :1153] = W2.astype(f16)

    b0s = np.tile(b0[None, :28], (128, 1)).astype(np.float32)
    # per-partition (drain) bias columns
    b1e = np.zeros((128, 2, KS), np.float32)          # b1[577 + kn2*9 + j]
    for t in range(2):
        b1e[:, t, :] = b1[577:].reshape(256, KS)[t * 128:(t + 1) * 128]
    b0e = (0.5 * b0[28:1180].reshape(128, KS)).astype(np.float32)  # [kn1, j]
    # free-varying (own-layer) bias patterns, flat (kd*9+j), part-replicated
    b2n = np.tile(b2[:1152].astype(f16)[None, :], (128, 1))
    b1n = np.tile((0.5 * b1[:576]).astype(f16)[None, :], (128, 1))

    return (W0own, W0x, W1o, W1x, W2p, b0s, b1e, b0e, b2n, b1n,
            float(b2[1152]), float(0.5 * b1[576]))


def kernel(x, W0, b0, W1, b1, W2, b2, _trace=False):
    from concourse import bass_utils

    x = np.asarray(x, np.float32)
    (W0own, W0x, W1o, W1x, W2p, b0s, b1e, b0e, b2n, b1n,
     b2_last, b1_last) = _prep_shared(
        np.asarray(W0, np.float32), np.asarray(b0, np.float32),
        np.asarray(W1, np.float32), np.asarray(b1, np.float32),
        np.asarray(W2, np.float32), np.asarray(b2, np.float32))

    if "nc" not in _CACHE:
        # bias-column constants are baked into the program as immediates
        _BIAS_CONSTS[0] = b2_last
        _BIAS_CONSTS[1] = b1_last
        _CACHE["nc"] = _build()
    nc = _CACHE["nc"]

    ident = np.eye(128, dtype=np.float16)

    # shard + transpose x on host: [B,448,512] -> per-core d-major fp16 layouts
    xs = x.reshape(N_CORES, BL, 448, D)
    in_maps = []
    for c in range(N_CORES):
        xc = xs[c]  # [BL, 448, 512]
        xt0 = np.ascontiguousarray(
            xc[:, 0:64, :].transpose(2, 0, 1)).reshape(4, 128, BL * 64).astype(
            np.float16)
        xt1 = np.ascontiguousarray(
            xc[:, 64:192, :].transpose(2, 0, 1)).reshape(4, 128, BL * 128).astype(
            np.float16)
        xt2 = np.ascontiguousarray(
            xc[:, 192:448, :].transpose(2, 0, 1)).reshape(4, 128, BL * 256).astype(
            np.float16)
        in_maps.append({
            "xt0": xt0, "xt1": xt1, "xt2": xt2,
            "w0": W0own, "w0x": W0x, "w1o": W1o, "w1x": W1x, "w2": W2p,
            "b0s": b0s, "b1e": b1e, "b0e": b0e, "b2n": b2n, "b1n": b1n,
            "ident": ident,
        })

    res = bass_utils.run_bass_kernel_spmd(
        nc, in_maps, core_ids=list(range(N_CORES)), trace=_trace)
    _CACHE["last_res"] = res
    y = np.concatenate([res.results[c]["y"] for c in range(N_CORES)], axis=0)
    return y


# revision 27
# speedup vs baseline: 1.3921x; 1.1663x over previous
"""Trainium2 Bass kernel for nn_DebedderNeuron (scatter_memory).

Strategy: data-parallel over batch (16 rows per core x 8 cores).
The scatter-add in the reference has closed-form structure:
  y[b] = concat(L0w, L0bias, 0.5*(L1w_own + L0ext^T), 0.5*L1bias,
                L2w_own + L1ext^T, L2bias)

v5 design:
- All matmuls fp16 (x and W converted host-side); PSUM accumulation f32.
- Own-slice GEMMs orientation A (out = [kernel-part, w-cols]) with
  9-aligned PSUM chunks; extension GEMMs orientation B
  (out = [next-kernel-part, (b, prev-kernel)]).
- Ext results land in tmp SBUF tiles stored FLAT j-minor
  ((b*K + kd)*9 + j) so the PE fold below streams them contiguously.
  Drains from ext PSUM add the partition-varying bias and do the
  j-interleave via strided writes, split across scalar ACT (h=0/even-j)
  and vector tensor_scalar (h=1/odd-j). The free-varying (own-layer)
  bias is one flat fp16 dual-pump vector add per phase.
- The own+ext combine is FOLDED INTO THE PE: an identity-stationary
  matmul accumulates the tmp tile into the own-GEMM PSUM (contiguous
  rhs stream), so no per-row vector combines exist at all. A contiguous
  scalar ACT copy drains PSUM->SBUF for the store DMA.
- Schedule is input-bandwidth aware: l01 (smallest inputs) runs first
  while the heavy w1x/w2/xt2 stream; 24 identity warmup matmuls flip
  HAM to K=8/8 during the load window. l01(1) is hoisted before the
  last own_l2 so the post-PE tail is one row's drain+store.
- Output stores are full-128-partition DMAs alternating sync/gpsimd.
"""
import sys

if '/opt/trn_rl_repo' not in sys.path:
    sys.path.insert(0, '/opt/trn_rl_repo')

import numpy as np

N_CORES = 8
B = 128
BL = B // N_CORES          # 16 batch rows per core
D = 512
KS = 9
I_DIM = 370816
# y layout offsets
OFF_L0W, OFF_L0B = 0, 1728
OFF_L1W, OFF_L1B = 1792, 75520
OFF_L2W, OFF_L2B = 75648, 370560

# 9-aligned own-GEMM chunking: (psum offset, w-col start, n cols, n kd)
L2_CHUNKS = ((0, 0, 504, 56), (512, 504, 504, 56), (1024, 1008, 144, 16))
L1_CHUNKS = ((0, 0, 504, 56), (512, 504, 72, 8))

_CACHE = {}
_BIAS_CONSTS = [0.0, 0.0]  # (b2[1152], 0.5*b1[576]) — set before _build()


def _build():
    import concourse.bacc as bacc
    import concourse.mybir as mybir
    import concourse.tile as tile

    F32 = mybir.dt.float32
    F16 = mybir.dt.float16
    IDENT = mybir.ActivationFunctionType.Identity

    nc = bacc.Bacc("TRN2", target_bir_lowering=False, debug=False)

    # x pre-transposed on host: [k(4), p(128), cols] fp16, cols b-major
    xt0_d = nc.dram_tensor("xt0", [4, 128, BL * 64], F16, kind="ExternalInput").ap()
    xt1_d = nc.dram_tensor("xt1", [4, 128, BL * 128], F16, kind="ExternalInput").ap()
    xt2_d = nc.dram_tensor("xt2", [4, 128, BL * 256], F16, kind="ExternalInput").ap()
    w0_d = nc.dram_tensor("w0", [D, 28], F16, kind="ExternalInput").ap()
    w0x_d = nc.dram_tensor("w0x", [D, KS * 128], F16, kind="ExternalInput").ap()
    w1o_d = nc.dram_tensor("w1o", [D, 578], F16, kind="ExternalInput").ap()
    w1x_d = nc.dram_tensor("w1x", [D, KS * 256], F16, kind="ExternalInput").ap()
    w2_d = nc.dram_tensor("w2", [D, 1154], F16, kind="ExternalInput").ap()
    ident_d = nc.dram_tensor("ident", [128, 128], F16, kind="ExternalInput").ap()
    b0s_d = nc.dram_tensor("b0s", [128, 28], F32, kind="ExternalInput").ap()
    # per-partition bias columns for the ext drains
    b1e_d = nc.dram_tensor("b1e", [128, 2, KS], F32, kind="ExternalInput").ap()
    b0e_d = nc.dram_tensor("b0e", [128, KS], F32, kind="ExternalInput").ap()
    # free-varying (own-layer) bias patterns, flat (kd*9+j), un-replicated
    b2n_d = nc.dram_tensor("b2n", [128, 1152], F16, kind="ExternalInput").ap()
    b1n_d = nc.dram_tensor("b1n", [128, 576], F16, kind="ExternalInput").ap()
    y_d = nc.dram_tensor("y", [BL, I_DIM], F32, kind="ExternalOutput").ap()

    with tile.TileContext(nc) as tc:
        with tc.tile_pool(name="const", bufs=1) as cp, \
             tc.tile_pool(name="xts", bufs=2) as xtp, \
             tc.tile_pool(name="xt1s", bufs=2) as xt1p, \
             tc.tile_pool(name="xt0p", bufs=1) as xt0p, \
             tc.tile_pool(name="tmp", bufs=1) as tmpp, \
             tc.tile_pool(name="y2s", bufs=3) as y2p, \
             tc.tile_pool(name="y1s", bufs=2) as y1p, \
             tc.tile_pool(name="l0s", bufs=2) as l0p, \
             tc.tile_pool(name="pso", bufs=2, space="PSUM") as psop, \
             tc.tile_pool(name="pse", bufs=2, space="PSUM") as psep:

            ident = cp.tile([128, 128], F16, tag="ident")
            w0 = cp.tile([128, 4, 28], F16, tag="w0")
            w0x = cp.tile([128, 4, KS * 128], F16, tag="w0x")
            w1o = cp.tile([128, 4, 578], F16, tag="w1o")
            w1x = cp.tile([128, 4, KS * 256], F16, tag="w1x")
            w2 = cp.tile([128, 4, 1154], F16, tag="w2")
            b0s = cp.tile([128, 28], F32, tag="b0s")
            b1e = cp.tile([128, 2, KS], F32, tag="b1e")
            b0e = cp.tile([128, KS], F32, tag="b0e")
            b2n = cp.tile([128, 1152], F16, tag="b2n")
            b1n = cp.tile([128, 576], F16, tag="b1n")
            # b-replicated flat bias tiles (built on-device, vector copies)
            b2r = cp.tile([128, 8, 1152], F16, tag="b2r")
            b1r = cp.tile([128, 8, 576], F16, tag="b1r")

            l0b_all = cp.tile([128, 8], F32, tag="l0b")      # [(q,kd), pair]
            l1b_all = cp.tile([128, 16], F32, tag="l1b")     # [kn1, b]
            l2b_all = cp.tile([128, 2, 16], F32, tag="l2b")  # [kn2, t, b]

            w1xr = w1x_d.rearrange("(a p) l -> p a l", p=128)
            w2r = w2_d.rearrange("(a p) l -> p a l", p=128)
            w0xr = w0x_d.rearrange("(a p) l -> p a l", p=128)

            engs3 = (nc.sync, nc.scalar, nc.gpsimd)
            xt0t = xt0p.tile([128, 4, 1024], F16, tag="xt0", name="xt_xt0")
            xt1t0 = xt1p.tile([128, 4, 1024], F16, tag="xt1", name="xt_xt1")

            # ident first (tiny), then PE warmup matmuls: dummy PE work
            # during the load window flips HAM to K=8/8 before real MMs
            nc.sync.dma_start(ident[:, :], ident_d)
            ps_w = psep.tile([128, 512], F32, tag="pse", name="ps_w")
            for _ in range(24):
                nc.tensor.matmul(ps_w[:, 0:128], ident, ident,
                                 start=True, stop=True)

            # --- loads ordered by phase order (l01(0) first: cheapest
            # inputs), round-robin across 3 issue queues ---
            i = 0
            # A: ext0-su0 minimal (w0x j-triple 0, xt0 su0 half)
            for k in range(4):
                engs3[i % 3].dma_start(w0x[:, k, 0:384],
                                       w0xr[:, k, 0:384]); i += 1
                engs3[i % 3].dma_start(xt0t[:, k, 0:512],
                                       xt0_d[k, :, 0:512]); i += 1
            nc.sync.dma_start(b0e[:, :], b0e_d)
            nc.scalar.dma_start(b1n[:, :], b1n_d)
            nc.gpsimd.dma_start(b0s[:, :], b0s_d)
            for h in range(8):
                nc.vector.tensor_copy(b1r[:, h, :], b1n[:, :])
            # A2: rest of w0x
            for k in range(4):
                engs3[i % 3].dma_start(w0x[:, k, 384:768],
                                       w0xr[:, k, 384:768]); i += 1
                engs3[i % 3].dma_start(w0x[:, k, 768:1152],
                                       w0xr[:, k, 768:1152]); i += 1
            # B: l0 + own-L1 su0 inputs
            for k in range(4):
                engs3[i % 3].dma_start(
                    w0[:, k, :],
                    w0_d.rearrange("(a p) l -> p a l", p=128)[:, k, :]); i += 1
                engs3[i % 3].dma_start(
                    w1o[:, k, :],
                    w1o_d.rearrange("(a p) l -> p a l", p=128)[:, k, :]); i += 1
                engs3[i % 3].dma_start(xt1t0[:, k, :],
                                       xt1_d[k, :, 0:1024]); i += 1
            # C: ext1 inputs (w1x) + its biases
            for k in range(4):
                for jc in range(3):
                    engs3[i % 3].dma_start(
                        w1x[:, k, jc * 768:(jc + 1) * 768],
                        w1xr[:, k, jc * 768:(jc + 1) * 768]); i += 1
            nc.sync.dma_start(b1e[:, :, :], b1e_d)
            nc.scalar.dma_start(b2n[:, :], b2n_d)
            for h in range(8):
                nc.vector.tensor_copy(b2r[:, h, :], b2n[:, :])

            def load_xt2(su, t):
                """[128, 4, 1024] = (8b x 128 kn2) for kn2-half t (strided)."""
                t_ = xtp.tile([128, 4, 1024], F16, tag="xt2", name="xt_xt2")
                for k in range(4):
                    eng = (nc.sync, nc.scalar)[k % 2]
                    eng.dma_start(
                        t_[:, k, :].rearrange("p (b c) -> p b c", c=128),
                        xt2_d[k, :, su * 2048:(su + 1) * 2048].rearrange(
                            "p (b c) -> p b c", c=256)[:, :, t * 128:(t + 1) * 128])
                return t_

            def load_xt1(su):
                t_ = xt1p.tile([128, 4, 1024], F16, tag="xt1", name="xt_xt1")
                for k in range(4):
                    eng = (nc.sync, nc.scalar)[k % 2]
                    eng.dma_start(t_[:, k, :], xt1_d[k, :, su * 1024:(su + 1) * 1024])
                return t_

            def ext1_phase(su, t, xt1t):
                """ext1 GEMMs for 8 rows, half t -> tmp2 flat (b,kd,j) fp16.

                h-outer loop: after each h-half's 9 contiguous ACT drains
                land in tmp2j, ONE 3D-AP op (strided read, sequential
                write) interleaves j and adds the free-varying bias,
                alternating vector/gpsimd so the two halves overlap.
                """
                tmp2 = tmpp.tile([128, 8, 128, KS], F16, tag="tmp2",
                                 name="tmp2", bufs=1)
                for h in range(2):
                    t2j = tmpp.tile([128, KS, 512], F16, tag="tmp2j",
                                    name="t2j", bufs=2)
                    for j in range(KS):
                        ps = psep.tile([128, 512], F32, tag="pse")
                        for k in range(4):
                            nc.tensor.matmul(
                                ps[:, :],
                                w1x[:, k, j * 256 + t * 128:j * 256 + t * 128 + 128],
                                xt1t[:, k, h * 512:(h + 1) * 512],
                                start=(k == 0), stop=(k == 3))
                        nc.scalar.activation(t2j[:, j, :], ps[:, :],
                                             IDENT, bias=b1e[:, t, j:j + 1])
                    nc.vector.tensor_add(
                        tmp2[:, h * 4:(h + 1) * 4, :, :].rearrange(
                            "p b kd j -> p (b kd) j"),
                        t2j.rearrange("p j bk -> p bk j"),
                        b2r[:, h * 4:(h + 1) * 4, :].rearrange(
                            "p b (kd j) -> p (b kd) j", j=KS))
                return tmp2

            def own_l2(su, t, xt2t, tmp2, b2_last):
                """own L2 GEMMs + PE-folded ext add + y2 DMA, 8 rows, half t."""
                t2f = tmp2.rearrange("p b kd j -> p (b kd j)")
                for bq in range(8):
                    gb = su * 8 + bq
                    y2sb = y2p.tile([128, 1154], F32, tag="y2s", name="y2sb")
                    pso = psop.tile([128, 1536], F32, tag="pso")
                    for k in range(4):
                        st_ = xt2t[:, k, bq * 128:(bq + 1) * 128]
                        for (po, c0, ncc, nkd) in L2_CHUNKS:
                            n = ncc + (2 if c0 == 1008 else 0)  # chunk3 incl bias col
                            nc.tensor.matmul(pso[:, po:po + n], st_,
                                             w2[:, k, c0:c0 + n],
                                             start=(k == 0), stop=False)
                    # fold ext+biases into PSUM: identity-stationary matmul
                    # streams the flat tmp2 slice (contiguous) accumulating
                    # onto the own-GEMM result — no vector combine at all
                    for ci, (po, c0, ncc, nkd) in enumerate(L2_CHUNKS):
                        nc.tensor.matmul(
                            pso[:, po:po + ncc], ident,
                            t2f[:, bq * 1152 + c0:bq * 1152 + c0 + ncc],
                            start=False, stop=(ci == 2),
                            skip_group_check=True)
                    # contiguous drains PSUM -> SBUF for the store
                    nc.scalar.copy(
                        y2sb[:, 0:1008].rearrange("p (c e) -> p c e", e=504),
                        pso[:, 0:1024].rearrange(
                            "p (c e) -> p c e", e=512)[:, :, 0:504])
                    nc.scalar.copy(y2sb[:, 1008:1154], pso[:, 1024:1170])
                    nc.vector.tensor_scalar_add(
                        l2b_all[:, t, gb:gb + 1], pso[:, 1168:1169], b2_last)
                    # one full-128-partition DMA (contiguous 576KB HBM span);
                    # alternate issue queues to parallelize descriptor gen
                    eng = (nc.sync, nc.gpsimd)[bq % 2]
                    eng.dma_start(
                        y_d[gb, OFF_L2W + t * 147456:
                            OFF_L2W + (t + 1) * 147456]
                        .rearrange("(kn w) -> kn w", w=1152),
                        y2sb[:, 0:1152])
                return

            def l01_phase(su, xt1t, b1_last):
                """ext0 + own L1 + own L0 for 8 rows."""
                # ext0: j loop, out [kn1, (8b, 64kd0)] -> tmp1 flat (b,kd,j)
                tmp1 = tmpp.tile([128, 8, 64, KS], F16, tag="tmp1", name="tmp1")
                t1j = tmpp.tile([128, KS, 512], F16, tag="tmp1j", name="t1j")
                for j in range(KS):
                    ps = psep.tile([128, 512], F32, tag="pse")
                    for k in range(4):
                        nc.tensor.matmul(ps[:, :],
                                         w0x[:, k, j * 128:(j + 1) * 128],
                                         xt0t[:, k, su * 512:(su + 1) * 512],
                                         start=(k == 0), stop=(k == 3))
                    nc.scalar.activation(t1j[:, j, :], ps[:, :],
                                         IDENT, bias=b0e[:, j:j + 1])
                nc.vector.tensor_add(
                    tmp1.rearrange("p b kd j -> p (b kd) j"),
                    t1j.rearrange("p j bk -> p bk j"),
                    b1r.rearrange("p b (kd j) -> p (b kd) j", j=KS))
                t1f = tmp1.rearrange("p b kd j -> p (b kd j)")
                # own L1 per row + PE-folded ext0 add
                for bq in range(8):
                    gb = su * 8 + bq
                    if bq % 4 == 0:
                        y1sb = y1p.tile([128, 4, 578], F32, tag="y1s", name="y1sb")
                    pso = psop.tile([128, 1536], F32, tag="pso")
                    for k in range(4):
                        st_ = xt1t[:, k, bq * 128:(bq + 1) * 128]
                        for (po, c0, ncc, nkd) in L1_CHUNKS:
                            n = ncc + (2 if c0 == 504 else 0)  # incl bias col + pad
                            nc.tensor.matmul(pso[:, po:po + n], st_,
                                             w1o[:, k, c0:c0 + n],
                                             start=(k == 0), stop=False)
                    for ci, (po, c0, ncc, nkd) in enumerate(L1_CHUNKS):
                        nc.tensor.matmul(
                            pso[:, po:po + ncc], ident,
                            t1f[:, bq * 576 + c0:bq * 576 + c0 + ncc],
                            start=False, stop=(ci == 1),
                            skip_group_check=True)
                    nc.scalar.copy(y1sb[:, bq % 4, 0:504], pso[:, 0:504])
                    nc.scalar.copy(y1sb[:, bq % 4, 504:576], pso[:, 512:584])
                    nc.vector.tensor_scalar_add(
                        l1b_all[:, gb:gb + 1], pso[:, 584:585], b1_last)
                    if bq % 4 == 3:
                        b0r = su * 8 + (bq - 3)
                        nc.gpsimd.dma_start(
                            y_d[b0r:b0r + 4, OFF_L1W:OFF_L1B]
                            .rearrange("b (kn w) -> kn b w", w=576),
                            y1sb[:, :, 0:576])
                # own L0 per pair of rows
                l0sb = l0p.tile([128, 4, 27], F32, tag="l0s", name="l0sb")
                for pair in range(4):
                    ps = psep.tile([128, 512], F32, tag="pse")
                    for k in range(4):
                        nc.tensor.matmul(
                            ps[:, 0:28],
                            xt0t[:, k, su * 512 + pair * 128:su * 512 + (pair + 1) * 128],
                            w0[:, k, 0:28],
                            start=(k == 0), stop=(k == 3))
                    nc.vector.tensor_add(l0sb[:, pair, :], ps[:, 0:27], b0s[:, 0:27])
                    gp = 4 * su + pair
                    nc.vector.tensor_add(l0b_all[:, gp:gp + 1], ps[:, 27:28],
                                         b0s[:, 27:28])
                for q in range(2):
                    nc.sync.dma_start(
                        y_d[su * 8:su * 8 + 8, OFF_L0W:OFF_L0B]
                        .rearrange("(p q) (kd w) -> q kd p w", q=2, w=27)[q],
                        l0sb[q * 64:(q + 1) * 64, :, :])

            # ---- bias-region epilogue: PE-transpose the collected bias
            # columns to row-major, then store with fat descriptors (the
            # old partition-transposing DMAs were ~300 tiny descriptors
            # and stalled the store ring for ~15us) ----
            def finals_all(ident32):
                bt = l0p.tile([128, 4, 128], F32, tag="bt", name="bt")
                for t in range(2):
                    ps = psep.tile([128, 512], F32, tag="pse", name="ps_f")
                    nc.tensor.matmul(ps[0:16, 0:128], l2b_all[:, t, :],
                                     ident32, start=True, stop=True)
                    nc.scalar.copy(bt[0:16, t, :], ps[0:16, 0:128])
                    nc.sync.dma_start(
                        y_d[:, OFF_L2B + t * 128:OFF_L2B + (t + 1) * 128],
                        bt[0:16, t, :])
                ps = psep.tile([128, 512], F32, tag="pse", name="ps_f")
                nc.tensor.matmul(ps[0:16, 0:128], l1b_all[:, :], ident32,
                                 start=True, stop=True)
                nc.scalar.copy(bt[0:16, 2, :], ps[0:16, 0:128])
                nc.sync.dma_start(y_d[:, OFF_L1B:OFF_L1B + 128],
                                  bt[0:16, 2, :])
                ps = psep.tile([128, 512], F32, tag="pse", name="ps_f")
                nc.tensor.matmul(ps[0:8, 0:128], l0b_all[:, :], ident32,
                                 start=True, stop=True)
                nc.scalar.copy(bt[0:8, 3, :], ps[0:8, 0:128])
                for su in range(2):
                    for q in range(2):
                        nc.sync.dma_start(
                            y_d[su * 8:su * 8 + 8, OFF_L0B:OFF_L1W]
                            .rearrange("(pair q) c -> q pair c", q=2)[q],
                            bt[su * 4:su * 4 + 4, 3, q * 64:(q + 1) * 64])

            # ---------------- main schedule ----------------
            b2_last, b1_last = _BIAS_CONSTS

            # su0's heavy inputs: issue before l01 so rings stream during it
            xt2t00 = load_xt2(0, 0)
            for k in range(4):
                eng = (nc.sync, nc.scalar)[k % 2]
                eng.dma_start(w2[:, k, :], w2r[:, k, :])
            xt2t01 = load_xt2(0, 1)
            # xt0 su1-half (needed only at l01(1))
            for k in range(4):
                engs3[k % 3].dma_start(xt0t[:, k, 512:1024],
                                       xt0_d[k, :, 512:1024])

            ident32 = cp.tile([128, 128], F32, tag="ident32")
            nc.vector.tensor_copy(ident32[:, :], ident[:, :])

            l01_phase(0, xt1t0, b1_last)
            tmp2 = ext1_phase(0, 0, xt1t0)
            xt1t1 = load_xt1(1)
            own_l2(0, 0, xt2t00, tmp2, b2_last)
            tmp2 = ext1_phase(0, 1, xt1t0)
            xt2t10 = load_xt2(1, 0)
            own_l2(0, 1, xt2t01, tmp2, b2_last)
            tmp2 = ext1_phase(1, 0, xt1t1)
            xt2t11 = load_xt2(1, 1)
            own_l2(1, 0, xt2t10, tmp2, b2_last)
            tmp2 = ext1_phase(1, 1, xt1t1)
            l01_phase(1, xt1t1, b1_last)
            own_l2(1, 1, xt2t11, tmp2, b2_last)
            finals_all(ident32)

    nc.compile()
    return nc


def _prep_shared(W0, b0, W1, b1, W2, b2):
    """Host-side prescale + bias tile construction (numpy, core-independent)."""
    f16 = np.float16
    W0own = W0[:, :28].astype(f16)
    # ext0 cols packed dense, j-major: w0x[:, j*128 + kn] = 0.5*W0[:, 28+kn*9+j]
    W0x = (0.5 * W0[:, 28:]).reshape(D, 128, KS).transpose(0, 2, 1).reshape(
        D, KS * 128).astype(f16)
    W1o = np.zeros((D, 578), f16)
    W1o[:, :577] = (0.5 * W1[:, :577]).astype(f16)
    # ext1 cols packed dense, j-major: w1x[:, j*256 + kn] = W1[:, 577+kn*9+j]
    W1x = W1[:, 577:].reshape(D, 256, KS).transpose(0, 2, 1).reshape(
        D, KS * 256).astype(f16)
    W2p = np.zeros((D, 1154), f16)
    W2p[:, :1153] = W2.astype(f16)

    b0s = np.tile(b0[None, :28], (128, 1)).astype(np.float32)
    # per-partition (drain) bias columns
    b1e = np.zeros((128, 2, KS), np.float32)          # b1[577 + kn2*9 + j]
    for t in range(2):
        b1e[:, t, :] = b1[577:].reshape(256, KS)[t * 128:(t + 1) * 128]
    b0e = (0.5 * b0[28:1180].reshape(128, KS)).astype(np.float32)  # [kn1, j]
    # free-varying (own-layer) bias patterns, flat (kd*9+j), part-replicated
    b2n = np.tile(b2[:1152].astype(f16)[None, :], (128, 1))
    b1n = np.tile((0.5 * b1[:576]).astype(f16)[None, :], (128, 1))

    return (W0own, W0x, W1o, W1x, W2p, b0s, b1e, b0e, b2n, b1n,
            float(b2[1152]), float(0.5 * b1[576]))


def kernel(x, W0, b0, W1, b1, W2, b2, _trace=False):
    from concourse import bass_utils

    x = np.asarray(x, np.float32)
    (W0own, W0x, W1o, W1x, W2p, b0s, b1e, b0e, b2n, b1n,
     b2_last, b1_last) = _prep_shared(
        np.asarray(W0, np.float32), np.asarray(b0, np.float32),
        np.asarray(W1, np.float32), np.asarray(b1, np.float32),
        np.asarray(W2, np.float32), np.asarray(b2, np.float32))

    if "nc" not in _CACHE:
        # bias-column constants are baked into the program as immediates
        _BIAS_CONSTS[0] = b2_last
        _BIAS_CONSTS[1] = b1_last
        _CACHE["nc"] = _build()
    nc = _CACHE["nc"]

    ident = np.eye(128, dtype=np.float16)

    # shard + transpose x on host: [B,448,512] -> per-core d-major fp16 layouts
    xs = x.reshape(N_CORES, BL, 448, D)
    in_maps = []
    for c in range(N_CORES):
        xc = xs[c]  # [BL, 448, 512]
        xt0 = np.ascontiguousarray(
            xc[:, 0:64, :].transpose(2, 0, 1)).reshape(4, 128, BL * 64).astype(
            np.float16)
        xt1 = np.ascontiguousarray(
            xc[:, 64:192, :].transpose(2, 0, 1)).reshape(4, 128, BL * 128).astype(
            np.float16)
        xt2 = np.ascontiguousarray(
            xc[:, 192:448, :].transpose(2, 0, 1)).reshape(4, 128, BL * 256).astype(
            np.float16)
        in_maps.append({
            "xt0": xt0, "xt1": xt1, "xt2": xt2,
            "w0": W0own, "w0x": W0x, "w1o": W1o, "w1x": W1x, "w2": W2p,
            "b0s": b0s, "b1e": b1e, "b0e": b0e, "b2n": b2n, "b1n": b1n,
            "ident": ident,
        })

    res = bass_utils.run_bass_kernel_spmd(
        nc, in_maps, core_ids=list(range(N_CORES)), trace=_trace)
    _CACHE["last_res"] = res
    y = np.concatenate([res.results[c]["y"] for c in range(N_CORES)], axis=0)
    return y
